# revision 21
# baseline (speedup 1.0000x reference)
"""BTT layer kernel for Trainium2 (8 NeuronCores, data-parallel over batch).

Computes y = BTT(x; W1, W2) where
  x: (4096, 4096) fp32, W1: (64, 64, 256) fp32, W2: (64, 256, 64) fp32
  stage 1: t[b, m2, n1, r] = sum_m1 x[b, m2, m1] * W1[m2, m1, n1*4+r]
  stage 2: y[b, n1, n2]   = sum_{m2, r} t[b, m2, n1, r] * W2[n1, m2*4+r, n2]

v9: the intermediate t never leaves the chip.  Stage-1 emits per-(m2, h)
PSUM tiles whose partition layout is p' = r*32 + nl (n1 = 32h + nl), chosen
so the BTT "transpose" decomposes into bank-local 32x32 blocks.  ACT/DVE
evict PSUM into a staging buffer E with b-pairs packed innermost, then DVE
StreamTranspose instructions (uint32-viewed pairs, 2 elem/cycle/lane) swap
nl <-> m2p in-place into the stage-2 operand layout t2[r*32+m2p][n1][b].
Stage-2 runs 64 K=256 matmuls (col-pair tile_position concurrency) straight
from SBUF.  HBM traffic is ~11 MB/core (bf16 x/W1, fp16 W2, fp16 y).

Self-contained: hardcodes all shapes; imports the Bass toolchain from
/opt/trn_rl_repo.
"""

import os
import re
import sys

import numpy as np

sys.path.insert(0, "/opt/trn_rl_repo")

import bass_rust  # noqa: E402
import concourse.bass as bass  # noqa: E402
import concourse.mybir as mybir  # noqa: E402
import concourse.tile as tile  # noqa: E402
from concourse import bass_utils  # noqa: E402

# ----------------------------------------------------------------------------
# Environment shims (same as v8)
# ----------------------------------------------------------------------------


def _install_walrus_single_wait_patch():
    """This container's walrus build supports only ONE sem-wait per
    instruction. TileContext attaches several (LDWEIGHTS after two DMAs, the
    kernel-tail drain). Split every multi-wait instruction: hoist all-but-one
    wait onto same-engine NoOps placed immediately before it, and emit the
    tail drain one proc at a time."""
    if getattr(tile.TileContext, "_single_wait_patched", False):
        return

    counter = [0]

    def _split_multiwait_insts(ordered):
        for insts in ordered.values():
            i = 0
            while i < len(insts):
                inst = insts[i]
                si = getattr(inst, "sync_info", None)
                if si is not None and len(si.on_wait) > 1:
                    waits = list(si.on_wait)
                    new_nops = []
                    for w in waits[:-1]:
                        counter[0] += 1
                        nop = mybir.InstNoOp(
                            name=f"waitsplit_{counter[0]}", ins=[], outs=[]
                        )
                        nop.engine = inst.engine
                        nop.sync_info = bass_rust.SyncInfo(on_wait=[w], on_update=[])
                        new_nops.append(nop)
                    inst.sync_info = bass_rust.SyncInfo(
                        on_wait=[waits[-1]], on_update=list(si.on_update)
                    )
                    insts[i:i] = new_nops
                    i += len(new_nops)
                i += 1

    orig_lower = tile.TileContext._lower_ordered_insts

    def patched_lower(self, ordered):
        _split_multiwait_insts(ordered)
        return orig_lower(self, ordered)

    def split_drain_and_barrier(self, tick_clock, wait_clock):
        gc = tick_clock.global_clock
        ticks = [int(x) for x in re.findall(r"\d+", repr(gc.copy()))]
        emitted = False
        for i, t in enumerate(ticks):
            if t > 0:
                vec = [0] * len(ticks)
                vec[i] = t
                drain_inst = self.nc.sync.drain()
                wait_clock.add_sem_waits(
                    drain_inst.ins,
                    bass_rust.ScopedClock({None: bass_rust.VectorClock(vec)}),
                )
                emitted = True
        if not emitted:
            self.nc.sync.drain()
        self.nc.all_engine_barrier()
        assert self.sems is not None
        popped = self.nc._tile_sem_poison_stack.pop()
        assert popped is self._sem_poison
        self.nc.clear_and_free_semaphores(list(self.sems.allocated().values()))
        self.nc.all_engine_barrier()

    tile.TileContext._lower_ordered_insts = patched_lower
    tile.TileContext._drain_and_barrier = split_drain_and_barrier
    tile.TileContext._single_wait_patched = True


def _install_ntff_hook():
    """Register the NTFF profiling hook (missing antenv.axon_hooks module in
    this image). Only needed when profiling; harmless otherwise."""
    import types

    if "antenv.axon_hooks" not in sys.modules:
        import antenv

        mod = types.ModuleType("antenv.axon_hooks")
        mod._hook = None
        mod.set_axon_ntff_profile_hook = lambda h: setattr(mod, "_hook", h)
        mod.get_axon_ntff_profile_hook = lambda: mod._hook
        sys.modules["antenv.axon_hooks"] = mod
        antenv.axon_hooks = mod
    m = sys.modules["antenv.axon_hooks"]
    if m._hook is None:
        try:
            from trn_agent_boot.trn_boot import _ntff_profile_via_ctypes

            m.set_axon_ntff_profile_hook(
                _ntff_profile_via_ctypes("/opt/axon/libaxon_pjrt.so")
            )
        except Exception:
            pass
    bass_utils.upload_artifacts = lambda d: d


_install_walrus_single_wait_patch()

# ----------------------------------------------------------------------------
# Problem constants / tunables
# ----------------------------------------------------------------------------

B = 4096
M1 = M2 = N1 = N2 = 64
R = 4
NCORES = 8
BP = B // NCORES  # batch rows per core (512)

BC = int(os.environ.get("BTT_BC", "256"))  # b-chunk
S1_DT = os.environ.get("BTT_S1_DT", "bfloat16")  # x / W1 dtype
T_DT = os.environ.get("BTT_T_DT", "float16")  # E / t2 / W2 dtype
Y_DT = os.environ.get("BTT_Y_DT", "float16")  # y output dtype
# eviction engine pattern: of every 8 evictions, this many go to DVE
DVE_EVICT = int(os.environ.get("BTT_DVE_EVICT", "3"))
XG = int(os.environ.get("BTT_XG", "8"))  # x-load group (g per DMA)
# of the 8 (chunk, j, h) transpose units, how many go via DRAM bounce
NDMA_T = int(os.environ.get("BTT_NDMA_T", "1"))
EV_WIDE = os.environ.get("BTT_EV_WIDE", "1") == "1"  # [128,1024] evictions
YV = os.environ.get("BTT_YV", "1") == "1"  # alternate y-evict on DVE
TSPLIT = int(os.environ.get("BTT_TSPLIT", "1"))  # transpose sub-instructions


def _np_of(dt_name):
    import ml_dtypes

    return {
        "float32": np.float32,
        "float32r": np.float32,
        "float16": np.float16,
        "bfloat16": ml_dtypes.bfloat16,
    }[dt_name]


# ----------------------------------------------------------------------------
# Bass program
# ----------------------------------------------------------------------------


def build_program(bc=None, s1_dt=None, t_dt=None, y_dt=None, dve_evict=None,
                  ndma_t=None):
    bc = bc or BC
    s1dt = getattr(mybir.dt, s1_dt or S1_DT)
    tdt = getattr(mybir.dt, t_dt or T_DT)
    ydt = getattr(mybir.dt, y_dt or Y_DT)
    dve_evict = DVE_EVICT if dve_evict is None else dve_evict
    ndma_t = NDMA_T if ndma_t is None else ndma_t
    nch = BP // bc
    hbc = bc // 2  # b-pairs per chunk
    f32 = mybir.dt.float32
    u32 = mybir.dt.uint32

    # assignment of (ci, j, h) transpose units to the DRAM-bounce path.
    # Prefer j=0 units: they are produced mid-chunk, so the bounce's extra
    # latency hides under the rest of stage 1.
    order = [(ci, j, h) for j in range(2) for h in range(2) for ci in range(nch)]
    dma_units = set(order[:ndma_t])

    nc = bass.Bass(
        "TRN2",
        target_bir_lowering=False,
        debug=False,
        detect_race_conditions=os.environ.get("BTT_NO_RACE", "0") != "1",
    )

    # Host-marshalled layouts (see _marshal_inputs):
    #   xt[p][ci][g][b'] = x[c*BP + ci*bc + b', m2*64 + m1]
    #       p = m1 + 64*(m2%2), g = m2//2
    #   w1[p][g][h][c']  = W1[2g + p//64, p%64, (32h + c'%32)*4 + c'//32]
    #       (c' = r*32 + nl encodes n1 = 32h + nl, r = c'//32)
    #   w2[k][n1][j][n2] = W2[n1, (32j + k%32)*4 + k//32, n2]   (k = r*32 + m2p)
    #   yt[p][ci][q][b'] = y[c*BP + ci*bc + b', (2q + p//64)*64 + p%64]
    xt_d = nc.dram_tensor("xt", [128, nch, 32, bc], s1dt, kind="ExternalInput")
    w1_d = nc.dram_tensor("w1", [128, 32, 2, 128], s1dt, kind="ExternalInput")
    w2_d = nc.dram_tensor("w2", [128, 64, 2, 64], tdt, kind="ExternalInput")
    yt_d = nc.dram_tensor("yt", [128, nch, 32, bc], ydt, kind="ExternalOutput")

    WG = 8  # w1 load group size

    with tile.TileContext(nc) as tc:
        with (
            tc.tile_pool(name="weights", bufs=1) as wpool,
            tc.tile_pool(name="xin", bufs=3) as xpool,
            tc.tile_pool(name="estage", bufs=2) as epool,
            tc.tile_pool(name="t2j0", bufs=2) as t2p0,
            tc.tile_pool(name="t2j1", bufs=1) as t2p1,
            tc.tile_pool(name="yout", bufs=2) as ypool,
            tc.tile_pool(name="dram", bufs=1, space="DRAM") as dram_pool,
            tc.tile_pool(name="ps1", bufs=3, space="PSUM") as ps1pool,
            tc.tile_pool(name="ps2", bufs=2, space="PSUM") as ps2pool,
        ):
            w1_sb = [
                wpool.tile([128, WG, 2, 128], s1dt, name=f"w1_sb{k}")
                for k in range(32 // WG)
            ]
            w2_sb = [
                wpool.tile([128, 32, 2, 64], tdt, name=f"w2_sb{k}") for k in range(2)
            ]

            def load_weights():
                for k in range(32 // WG):
                    nc.gpsimd.dma_start(
                        w1_sb[k][:], w1_d[:, k * WG : (k + 1) * WG, :, :]
                    )
                for k in range(2):
                    nc.gpsimd.dma_start(
                        w2_sb[k][:], w2_d[:, k * 32 : (k + 1) * 32, :, :]
                    )

            weights_loaded = [False]
            t2_of = {}
            e_tiles = {}
            xg_ref = [None]
            ps2_ref = [None]
            ysb_ref = [None]
            ectr = [0]
            yctr = [0]

            pending_t = []

            def drain_t(n=1):
                for _ in range(min(n, len(pending_t))):
                    pending_t.pop(0)()

            def evict(dst_ap, src_ap):
                if not pending_t and ectr[0] % 8 < dve_evict:
                    nc.vector.tensor_copy(dst_ap, src_ap)
                else:
                    nc.scalar.copy(dst_ap, src_ap)
                ectr[0] += 1

            def s1_step(ci, g):
                j = g // 16
                if g % XG == 0:
                    xg_ref[0] = xpool.tile([128, XG, bc], s1dt, tag="xg",
                                           name="xg")
                    nc.sync.dma_start(xg_ref[0][:], xt_d[:, ci, g : g + XG, :])
                    if not weights_loaded[0]:
                        load_weights()
                        weights_loaded[0] = True
                xg = xg_ref[0]
                if g % 16 == 0:
                    for h in range(2):
                        if (ci, j, h) in dma_units:
                            e_tiles[(j, h)] = epool.tile(
                                [128, 32, bc], tdt,
                                name=f"E2_{ci}_{j}_{h}", tag=f"E_{h}",
                            )
                        else:
                            e_tiles[(j, h)] = epool.tile(
                                [128, hbc, 32, 2], tdt,
                                name=f"E_{ci}_{j}_{h}", tag=f"E_{h}",
                            )
                    t2_of[(ci, j)] = (t2p0 if j == 0 else t2p1).tile(
                        [128, 64, bc], tdt, name=f"t2_{ci}_{j}",
                        tag=f"t2j{j}",
                    )
                ps = ps1pool.tile([128, 2, 2, bc], f32, tag="ps1", name="ps")
                for p in range(2):
                    for h in range(2):
                        nc.tensor.matmul(
                            ps[:, p, h, :],
                            w1_sb[g // WG][64 * p : 64 * p + 64, g % WG, h, :],
                            xg[64 * p : 64 * p + 64, g % XG, :],
                            start=True,
                            stop=True,
                        )
                m0 = 2 * (g % 16)
                for h in range(2):
                    if (ci, j, h) in dma_units:
                        dst_ap = e_tiles[(j, h)][:, m0 : m0 + 2, :]
                    else:
                        dst_ap = e_tiles[(j, h)][
                            :, :, m0 : m0 + 2, :
                        ].rearrange("k bp m bi -> k m bp bi")
                    evict(dst_ap, ps[:, :, h, :])
                if g % 16 == 15:
                    t2t = t2_of[(ci, j)]
                    for h in range(2):
                        if (ci, j, h) in dma_units:
                            td = dram_pool.tile(
                                [128, 32, bc], tdt, name=f"td_{ci}_{j}_{h}"
                            )
                            tdw = td[:].rearrange("(nl r) m b -> r nl m b", r=R)
                            for r in range(R):
                                nc.gpsimd.dma_start(
                                    tdw[r],
                                    e_tiles[(j, h)][32 * r : 32 * r + 32],
                                )
                            srcr = td[:].rearrange(
                                "(nl r) m b -> r m nl b", r=R
                            )
                            n0 = 32 * h
                            nc.gpsimd.dma_start(
                                t2t[:, n0 : n0 + 16, :], srcr[:, :, 0:16, :]
                            )
                            nc.gpsimd.dma_start(
                                t2t[:, n0 + 16 : n0 + 32, :],
                                srcr[:, :, 16:32, :],
                            )
                        else:
                            ein = e_tiles[(j, h)][:].bitcast(u32)
                            tu = t2t[:, 32 * h : 32 * h + 32, :].bitcast(u32)
                            nsp = TSPLIT
                            qb = hbc // nsp
                            for sq in range(nsp):
                                bp0 = sq * qb
                                pending_t.append(
                                    (lambda ei, to: lambda:
                                        nc.vector.transpose(to, ei))(
                                        ein[:, bp0 : bp0 + qb, :],
                                        tu[:, :, bp0 : bp0 + qb].rearrange(
                                            "k nl bp -> k bp nl"
                                        ),
                                    )
                                )

            def s2_step(ci, q):
                if q % 2 == 0:
                    ps2_ref[0] = ps2pool.tile([128, 2, bc], f32, tag="ps2",
                                              name="ps2")
                    ysb_ref[0] = ypool.tile([128, 2, bc], ydt, tag="ysb",
                                            name="ysb")
                ps2, ysb = ps2_ref[0], ysb_ref[0]
                for p in range(2):
                    n1 = 2 * q + p
                    for j in range(2):
                        nc.tensor.matmul(
                            ps2[64 * p : 64 * p + 64, q % 2, :],
                            w2_sb[n1 // 32][:, n1 % 32, j, :],
                            t2_of[(ci, j)][:, n1, :],
                            start=(j == 0),
                            stop=(j == 1),
                            tile_position=(0, 64 * p),
                        )
                if q % 2 == 1:
                    if YV and yctr[0] % 2 == 1:
                        nc.vector.tensor_copy(ysb[:], ps2[:])
                    else:
                        nc.scalar.copy(ysb[:], ps2[:])
                    yctr[0] += 1
                    nc.gpsimd.dma_start(yt_d[:, ci, q - 1 : q + 1, :], ysb[:])

            # software pipeline: stage-2 of chunk ci-1 interleaves with
            # stage-1 of chunk ci
            for ci in range(nch):
                for k in range(32):
                    s1_step(ci, k)
                    drain_t()
                    if ci > 0:
                        s2_step(ci - 1, k)
            for k in range(32):
                drain_t()
                s2_step(nch - 1, k)
            drain_t(99)

    return nc


# ----------------------------------------------------------------------------
# Host marshalling
# ----------------------------------------------------------------------------


def _marshal_inputs(x, W1, W2, s1_np, t_np, bc):
    nch = BP // bc
    # x: (B, 4096) -> xt[p][ci][g][b']: p = m1 + 64*(m2%2), g = m2//2
    xr = x.reshape(B, 32, 2, 64)  # (b, g, par, m1)
    xt_all = np.ascontiguousarray(
        xr.transpose(2, 3, 1, 0).reshape(128, 32, B)
    ).astype(s1_np, copy=False)  # [par*64+m1][g][b]
    # W1 (64, 64, 256) -> w1[p][g][h][c']: c' = r*32 + nl, n1 = 32h+nl
    w1r = W1.reshape(32, 2, 64, 2, 32, 4)  # [g][par][m1][h][nl][r]
    w1 = np.ascontiguousarray(
        w1r.transpose(1, 2, 0, 3, 5, 4).reshape(128, 32, 2, 128)
    ).astype(s1_np, copy=False)
    # W2 (64, 256, 64) -> w2[k][n1][j][n2]: k = r*32 + m2p
    w2r = W2.reshape(64, 2, 32, 4, 64)  # [n1][j][m2p][r][n2]
    w2 = np.ascontiguousarray(
        w2r.transpose(3, 2, 0, 1, 4).reshape(128, 64, 2, 64)
    ).astype(t_np, copy=False)

    in_maps = []
    for c in range(NCORES):
        xc = xt_all[:, :, c * BP : (c + 1) * BP]  # (128, 32, BP)
        xc = np.ascontiguousarray(
            xc.reshape(128, 32, nch, bc).transpose(0, 2, 1, 3)
        )  # [p][ci][g][b']
        in_maps.append({"xt": xc, "w1": w1, "w2": w2})
    return in_maps


def _unmarshal_output(results, bc):
    nch = BP // bc
    y = np.empty((B, N1 * N2), np.float32)
    for c, res in enumerate(results):
        yt = res["yt"]  # [p][ci][q][b'], p = n2 + 64*(n1%2), q = n1//2
        yc = (
            yt.astype(np.float32)
            .reshape(2, 64, nch, 32, bc)
            .transpose(2, 4, 3, 0, 1)
            .reshape(BP, 4096)
        )
        y[c * BP : (c + 1) * BP] = yc
    return y


# ----------------------------------------------------------------------------
# Public entry point
# ----------------------------------------------------------------------------

_PROGRAM_CACHE = {}


def kernel(x, W1, W2, _trace=False, _config=None):
    cfg = _config or {}
    key = tuple(sorted(cfg.items())) if cfg else None
    if key not in _PROGRAM_CACHE:
        _PROGRAM_CACHE[key] = build_program(**cfg)
    nc = _PROGRAM_CACHE[key]

    s1_np = _np_of(cfg.get("s1_dt", S1_DT))
    t_np = _np_of(cfg.get("t_dt", T_DT))
    bc = cfg.get("bc", BC)
    in_maps = _marshal_inputs(
        np.asarray(x, np.float32),
        np.asarray(W1, np.float32),
        np.asarray(W2, np.float32),
        s1_np,
        t_np,
        bc,
    )
    if _trace:
        _install_ntff_hook()
        os.environ["BASS_PERFETTO_PROFILE_ALL_CORES"] = "1"
    res = bass_utils.run_bass_kernel_spmd(
        nc, in_maps, core_ids=list(range(NCORES)), trace=_trace
    )
    y = _unmarshal_output(res.results, bc)
    if _trace:
        return y, res
    return y
